# revision 53
# baseline (speedup 1.0000x reference)
"""Multi-head causal attention (B=4, S=2048, D=1024, 16 heads) on 8 TRN2 cores.

Sharding: core c -> (batch b = c//2, head-group g = c%2). Each core computes
8 heads of one batch element end-to-end (QKV proj, causal softmax attention,
out-proj rows for its head slice). Host sums the two head-group partials per
batch and adds the output bias.

v3 (bf16 core, flipped PV):
  QT/KT[t] = (x @ w)^T per head-pair t; V into vo rows [V_h0|1|V_h1|1|...].
  scores ST[k, q-span] per pair of k-blocks into one [128,1024] psum (exact
  causal spans per block); one exp per pair on ACT; causal tri-masks on the
  diagonal sub-blocks (DVE).
  PV is FLIPPED: ctxT[q, 65] = sum_kb (P_kb block)^T [V_kb|1] with lhsT=P,
  so each matmul streams only 65 columns (4.3x fewer PE cycles than the
  [d, q] orientation) and the softmax denominator lands in ctxT col 64.
  Fully-masked (q-block, k-block) combinations are skipped outright. The 4
  q-block regions share one psum bank, and a start=True matmul pending-
  zeroes the whole 2KB bank, so the tile is cleared by one zero-weight
  matmul and all PV matmuls accumulate with start=False.
  Normalization: ctxT denominators -> [128,4,1] reciprocal (DVE), then a
  per-partition tensor_scalar_mul writes normalized ctx^T per q-block into
  ctn[128 q, 512 d]; a DMA xbar transpose turns that into
  cxt[128 d-part, 4 d-tile, q] for the bf16 out-projection.
  Projections for q-chunk qc+1 are interleaved into qc's attention as PE
  filler micro-ops (2-chunk matmul granularity); all out-projs are deferred
  into the ACT-heavy final chunk, and the final out-proj copies run on the
  then-idle ACT engine. Input DMAs are laddered on the SP queue so the
  first projection is gated by ~0.3MB of transfers; weights/x go bf16.
"""

import numpy as np
import ml_dtypes

B, S, D = 4, 2048, 1024
H_TOT = 16
HD = 64
NCORES = 8
GH = 8          # heads per core
GD = GH * HD    # 512: dout slice per core
NKB = S // 128  # 16 k-blocks
NQC = S // 512  # 4 q-chunks
BF16 = ml_dtypes.bfloat16

_cache = {}


def _build_body(tc, nc, mybir, xt_d, wq_d, wk_d, wv_d, ow_d, outp, dbg=None):
    from concourse.masks import make_upper_triangular
    import contextlib
    mybir_mod = mybir

    dt = mybir.dt
    F = mybir.ActivationFunctionType
    bf = dt.bfloat16

    pools = contextlib.ExitStack()
    tc_pool = lambda **kw: pools.enter_context(tc.tile_pool(**kw))

    singles = tc_pool(name="singles", bufs=1)
    pt_pool = tc_pool(name="pt", bufs=8)
    ctf_pool = tc_pool(name="ctf", bufs=3)
    rc_pool = tc_pool(name="rc", bufs=4)
    scp_pool = tc_pool(name="scp", bufs=3)
    ctn_pool = tc_pool(name="ctn", bufs=8)
    ost_pool = tc_pool(name="ost", bufs=6)
    psum_st = tc_pool(name="psum_st", bufs=2, space="PSUM")
    psum_ctx = tc_pool(name="psum_ctx", bufs=2, space="PSUM")
    psum_mm = tc_pool(name="psum_mm", bufs=2, space="PSUM")

    # ---- persistent SBUF tensors ----
    xt = singles.tile([128, 8, S], bf, name="xt")
    wq = singles.tile([128, 8, GD], bf, name="wq")
    wk = singles.tile([128, 8, GD], bf, name="wk")
    wv = singles.tile([128, 8, GD], bf, name="wv")
    ow = singles.tile([128, 4, D], bf, name="ow")
    qt = [singles.tile([128, S], bf, name=f"qt{t}") for t in range(4)]
    kt = [singles.tile([128, S], bf, name=f"kt{t}") for t in range(4)]
    VW = GH * 65    # [V_h|1] per head
    vo = singles.tile([128, NKB, VW], bf, name="vo")
    # cxt: (p, t, q) = ctx_norm[t*128+p, q]  (filled by DMA transposes)
    cxt = singles.tile([128, 4, S], bf, name="cxt")
    tri = singles.tile([128, 128], bf)   # keep k<=q
    zero128 = singles.tile([128, 128], bf, name="z128")
    make_upper_triangular(nc, tri, val=1.0, diag=True)
    nc.vector.memset(zero128, 0.0)
    nc.vector.memset(
        vo.rearrange("p k (h e) -> p k h e", e=65)[:, :, :, 64:65], 1.0)
    # exp(s/8) == pow(e^(1/8), s): constant base tile for the Pool-engine
    # exps that offload the ACT engine in the late, ACT-heavy chunks
    import math as _math
    cbase = singles.tile([128, 1024], dt.float32, name="cbase")
    nc.vector.memset(cbase, float(_math.exp(0.125)))

    # ---- input DMAs ----------------------------------------------------
    xt_r = xt_d.ap().rearrange("(t p) s -> p t s", p=128)
    wq_r = wq_d.ap().rearrange("(t p) n -> p t n", p=128)
    wk_r = wk_d.ap().rearrange("(t p) n -> p t n", p=128)
    wv_r = wv_d.ap().rearrange("(t p) n -> p t n", p=128)
    ow_r = ow_d.ap().rearrange("(t p) n -> p t n", p=128)
    nc.sync.dma_start(out=xt[:, 0:2, 0:512], in_=xt_r[:, 0:2, 0:512])
    nc.sync.dma_start(out=wk[:, :, 0:128], in_=wk_r[:, :, 0:128])
    nc.sync.dma_start(out=xt[:, 2:4, 0:512], in_=xt_r[:, 2:4, 0:512])
    nc.sync.dma_start(out=wq[:, :, 0:128], in_=wq_r[:, :, 0:128])
    nc.sync.dma_start(out=xt[:, 4:8, 0:512], in_=xt_r[:, 4:8, 0:512])
    nc.sync.dma_start(out=wv, in_=wv_r)
    nc.sync.dma_start(out=wk[:, :, 128:256], in_=wk_r[:, :, 128:256])
    nc.sync.dma_start(out=wq[:, :, 128:256], in_=wq_r[:, :, 128:256])
    nc.sync.dma_start(out=wk[:, :, 256:512], in_=wk_r[:, :, 256:512])
    nc.sync.dma_start(out=wq[:, :, 256:512], in_=wq_r[:, :, 256:512])
    nc.sync.dma_start(out=xt[:, :, 512:1024], in_=xt_r[:, :, 512:1024])
    nc.sync.dma_start(out=xt[:, :, 1024:S], in_=xt_r[:, :, 1024:S])
    nc.sync.dma_start(out=ow, in_=ow_r)

    # ---- PE work units (lists of micro-ops for fine-grain filling) -----
    def _proj_micros(lhsT_of, rhs_of, copy_fn):
        cell = {}

        def mm(c0):
            if c0 == 0:
                cell["ps"] = psum_mm.tile([128, 512], dt.float32, name="mmps")
            for c in range(c0, c0 + 2):
                nc.tensor.matmul(cell["ps"], lhsT=lhsT_of(c), rhs=rhs_of(c),
                                 start=(c == 0), stop=(c == 7))
            if c0 == 6:
                copy_fn(cell["ps"])
        return [(lambda c0=c0: mm(c0)) for c0 in range(0, 8, 2)]

    def proj_qk(w_sb, t_sb, t, qc):
        q0 = 512 * qc
        return _proj_micros(
            lambda c: w_sb[:, c, t * 128:(t + 1) * 128],
            lambda c: xt[:, c, q0:q0 + 512],
            lambda ps: nc.vector.tensor_copy(out=t_sb[t][:, q0:q0 + 512],
                                             in_=ps))

    def proj_v(kb):
        return _proj_micros(
            lambda c: xt[:, c, kb * 128:(kb + 1) * 128],
            lambda c: wv[:, c, :],
            lambda ps: nc.vector.tensor_copy(
                out=vo[:, kb, :].rearrange("p (h e) -> p h e", e=65)[:, :, 0:64],
                in_=ps.rearrange("p (h e) -> p h e", e=64)))

    def out_proj(sq, tail=False):
        """out rows [128sq,128sq+128) x all 1024 cols; one [128,1024] store."""
        cell = {}

        def oc_unit(oc):
            ps = psum_mm.tile([128, 512], dt.float32, name="mmps")
            for c in range(4):
                nc.tensor.matmul(
                    ps, lhsT=cxt[:, c, sq * 128:(sq + 1) * 128],
                    rhs=ow[:, c, oc * 512:(oc + 1) * 512],
                    start=(c == 0), stop=(c == 3))
            if oc == 0:
                cell["ost"] = ost_pool.tile([128, 1024], bf, name="ost")
            if tail and oc == 0:
                nc.scalar.copy(out=cell["ost"][:, 0:512], in_=ps)
            else:
                nc.vector.tensor_copy(out=cell["ost"][:, oc * 512:(oc + 1) * 512],
                                      in_=ps)
            if oc == 1:
                nc.sync.dma_start(
                    out=outp.ap()[sq * 128:(sq + 1) * 128, :], in_=cell["ost"])
        return [(lambda oc=oc: oc_unit(oc)) for oc in range(2)]

    ctn = {}  # (qc, j) -> staging tile [128 q, 512 d] bf16

    def emit_attn_head(qc, h, fillers):
        """scores+exp+mask for head h / chunk qc; flipped PV trails 2 pairs."""
        t, p0 = h // 2, (h % 2) * 64
        q0 = 512 * qc
        npairs = 2 * qc + 2
        ctxT = psum_ctx.tile([128, 4, 65], dt.float32, name="ctxT")
        # a start=True matmul wipes (pending-zeroes) its whole 2KB psum bank,
        # so zero the full 4-region tile with ONE zero-weight matmul up front
        # and accumulate every real PV matmul with start=False.
        nc.tensor.matmul(ctxT.rearrange("p j e -> p (j e)"), lhsT=zero128,
                         rhs=vo[:, 0, 0:260], start=True, stop=True,
                         skip_group_check=True)
        pend = []

        def emit_pv(p, n, pt):
            for i, kb in ((0, 2 * p), (1, 2 * p + 1)):
                for j in range(4):
                    if 128 * j + 127 < 128 * kb - q0:  # fully masked
                        continue
                    lo = i * 512 + 128 * j
                    if 128 * j < 512 - n:              # outside computed span
                        continue
                    nc.tensor.matmul(
                        ctxT[:, j, :],
                        lhsT=pt[:, lo:lo + 128],
                        rhs=vo[:, kb, 65 * h:65 * h + 65],
                        start=False, stop=(kb == 4 * qc + j),
                        skip_group_check=True)

        for p in range(npairs):
            n = 512 if p < npairs - 1 else 256
            stp = psum_st.tile([128, 1024], dt.float32, name="stp")
            for i, kb in ((0, 2 * p), (1, 2 * p + 1)):
                nb = min(n, 512 - max(0, 128 * kb - q0))  # exact causal span
                nc.tensor.matmul(
                    stp[:, (i + 1) * 512 - nb:(i + 1) * 512],
                    lhsT=kt[t][p0:p0 + 64, kb * 128:(kb + 1) * 128],
                    rhs=qt[t][p0:p0 + 64, q0 + 512 - nb:q0 + 512],
                    start=True, stop=True)
            pt = pt_pool.tile([128, 1024], bf, name="pt")
            ptv = pt.rearrange("p (k n) -> p k n", k=2)
            stv = stp.rearrange("p (k n) -> p k n", k=2)
            on_pool = False
            if on_pool:  # full pair: contiguous [128, 1024]
                scp = scp_pool.tile([128, 1024], dt.float32, name="scp")
                nc.vector.tensor_copy(out=scp, in_=stp)
                nc.gpsimd.tensor_tensor(out=pt, in0=cbase, in1=scp,
                                        op=mybir.AluOpType.pow)
            else:
                nc.scalar.activation(out=ptv[:, :, 512 - n:512],
                                     in_=stv[:, :, 512 - n:512],
                                     func=F.Exp, scale=0.125)
            if p >= npairs - 2:  # diagonal pair: tri-mask both blocks
                lo = 512 - n
                nc.vector.tensor_mul(pt[:, lo:lo + 128],
                                     pt[:, lo:lo + 128], tri)
                nc.vector.tensor_mul(pt[:, 512 + lo + 128:512 + lo + 256],
                                     pt[:, 512 + lo + 128:512 + lo + 256], tri)
            pend.append((p, n, pt))
            if len(pend) > 2:
                emit_pv(*pend.pop(0))
            if fillers:
                k = (len(fillers) + npairs - 1 - p) // (npairs - p)
                for u in fillers[:k]:
                    u()
                fillers = fillers[k:]
        for u in fillers:  # leftovers BEFORE the PV flush (PV may need them)
            u()
        for pp in pend:
            emit_pv(*pp)
        # ctxT -> sbuf, then per-q-row normalize into ctn staging
        ctf = ctf_pool.tile([128, 4, 65], dt.float32, name="ctf")
        nc.vector.tensor_copy(out=ctf, in_=ctxT)
        if dbg and qc == 0 and h == 0:
            nc.sync.dma_start(out=dbg["ctf00"].ap(), in_=ctf)
        rt = rc_pool.tile([128, 4, 1], dt.float32, name="rt")
        nc.vector.reciprocal(out=rt, in_=ctf[:, :, 64:65])
        for j in range(4):
            nc.vector.tensor_scalar_mul(
                ctn[(qc, j)][:, 64 * h:64 * h + 64],
                ctf[:, j, 0:64], rt[:, j, :])

    # ---- emission schedule ---------------------------------------------
    for m in proj_qk(wk, kt, 0, 0) + proj_qk(wq, qt, 0, 0):
        m()
    for kb in range(4):
        for m in proj_v(kb):
            m()

    def qc_fillers(qc):
        micros = []
        if qc == 0:
            for t in range(1, 4):
                micros += proj_qk(wk, kt, t, 0)
                micros += proj_qk(wq, qt, t, 0)
        if qc > 0:  # transpose last chunk's ctn into cxt (DMA xbar)
            for j in range(4):
                qb = 4 * (qc - 1) + j
                micros.append(
                    (lambda qcm=qc - 1, j=j, qb=qb: nc.sync.dma_start_transpose(
                        out=cxt[:, :, qb * 128:(qb + 1) * 128],
                        in_=ctn[(qcm, j)])))
        if qc + 1 < NQC:
            for t in range(4):
                micros += proj_qk(wk, kt, t, qc + 1)
            for kb in range(4 * qc + 4, 4 * qc + 8):
                micros += proj_v(kb)
            for t in range(4):
                micros += proj_qk(wq, qt, t, qc + 1)
        if qc == 3:  # all deferred out-projs land in the ACT-heavy tail
            for sq in range(0, 12):
                micros += out_proj(sq)
        return micros

    for qc in range(NQC):
        if qc == 1 and dbg:
            nc.sync.dma_start(out=dbg["ctn00"].ap(), in_=ctn[(0, 0)])
        for j in range(4):
            ctn[(qc, j)] = ctn_pool.tile([128, 512], bf, name="ctn")
        fillers = qc_fillers(qc)
        start_h = 2 if qc == 3 else 0   # hold qc3 fillers for the late heads
        done = 0
        for h in range(GH):
            if h < start_h:
                emit_attn_head(qc, h, [])
                continue
            left = GH - h
            share = (len(fillers) + left - 1) // left if fillers else 0
            mine, fillers = fillers[:share], fillers[share:]
            emit_attn_head(qc, h, mine)
        for u in fillers:
            u()
    for j in range(4):  # final chunk's transposes
        qb = 12 + j
        nc.sync.dma_start_transpose(out=cxt[:, :, qb * 128:(qb + 1) * 128],
                                    in_=ctn[(3, j)])
    for sq in range(12, 16):
        for m in out_proj(sq, tail=True):
            m()
    if dbg:
        nc.sync.dma_start(out=dbg["qt0"].ap(), in_=qt[0])
        nc.sync.dma_start(out=dbg["kt0"].ap(), in_=kt[0])
        nc.sync.dma_start(out=dbg["vo"].ap(), in_=vo)
        nc.sync.dma_start(out=dbg["cxt"].ap(), in_=cxt)

    return pools


def _build_nc():
    import concourse.tile as tile
    from concourse import bacc, mybir

    dt = mybir.dt
    nc = bacc.Bacc("TRN2", target_bir_lowering=False, debug=False,
                   num_devices=NCORES)
    xt_d = nc.dram_tensor("xt", [D, S], dt.bfloat16, kind="ExternalInput")
    wq_d = nc.dram_tensor("wq", [D, GD], dt.bfloat16, kind="ExternalInput")
    wk_d = nc.dram_tensor("wk", [D, GD], dt.bfloat16, kind="ExternalInput")
    wv_d = nc.dram_tensor("wv", [D, GD], dt.bfloat16, kind="ExternalInput")
    ow_d = nc.dram_tensor("ow", [GD, D], dt.bfloat16, kind="ExternalInput")
    outp = nc.dram_tensor("outp", [S, D], dt.bfloat16, kind="ExternalOutput")
    import os as _os
    dbg = None
    if _os.environ.get("KDEBUG"):
        dbg = {
            "qt0": nc.dram_tensor("dqt0", [128, S], dt.bfloat16, kind="ExternalOutput"),
            "kt0": nc.dram_tensor("dkt0", [128, S], dt.bfloat16, kind="ExternalOutput"),
            "vo": nc.dram_tensor("dvo", [128, NKB, GH * 65], dt.bfloat16, kind="ExternalOutput"),
            "cxt": nc.dram_tensor("dcxt", [128, 4, S], dt.bfloat16, kind="ExternalOutput"),
            "ctn00": nc.dram_tensor("dctn00", [128, 512], dt.bfloat16, kind="ExternalOutput"),
            "ctf00": nc.dram_tensor("dctf00", [128, 4, 65], dt.float32, kind="ExternalOutput"),
        }

    with tile.TileContext(nc) as tc:
        pools = _build_body(tc, nc, mybir, xt_d, wq_d, wk_d, wv_d, ow_d, outp, dbg)
        pools.close()
    nc.compile()
    return nc


LAST_RESULTS = None


def kernel(batch, w_query, w_key, w_value, out_w, out_b):
    global LAST_RESULTS
    import os
    from concourse import bass_utils

    try:  # BASS_TRACE needs the axon NTFF hook; without it the run crashes
        from antenv.axon_hooks import get_axon_ntff_profile_hook  # noqa: F401
    except ImportError:
        os.environ.setdefault("BASS_NEVER_TRACE", "1")

    batch = np.asarray(batch, dtype=np.float32)
    w_query = np.asarray(w_query, dtype=np.float32)
    w_key = np.asarray(w_key, dtype=np.float32)
    w_value = np.asarray(w_value, dtype=np.float32)
    out_w = np.asarray(out_w, dtype=np.float32)
    out_b = np.asarray(out_b, dtype=np.float32)

    if "nc" not in _cache:
        _cache["nc"] = _build_nc()
    nc = _cache["nc"]

    xts = [np.ascontiguousarray(batch[b].T).astype(BF16) for b in range(B)]
    slc = [slice(g * GD, (g + 1) * GD) for g in range(2)]
    wqs = [np.ascontiguousarray(w_query[:, s]).astype(BF16) for s in slc]
    wks = [np.ascontiguousarray(w_key[:, s]).astype(BF16) for s in slc]
    wvs = [np.ascontiguousarray(w_value[:, s]).astype(BF16) for s in slc]
    ows = [np.ascontiguousarray(out_w[s, :]).astype(BF16) for s in slc]
    in_maps = []
    for c in range(NCORES):
        b, g = divmod(c, 2)
        in_maps.append({
            "xt": xts[b], "wq": wqs[g], "wk": wks[g],
            "wv": wvs[g], "ow": ows[g],
        })

    res = bass_utils.run_bass_kernel_spmd(
        nc, in_maps, core_ids=list(range(NCORES)),
    )
    LAST_RESULTS = res

    out = np.empty((B, S, D), np.float32)
    for b in range(B):
        out[b] = res.results[2 * b]["outp"].astype(np.float32) \
            + res.results[2 * b + 1]["outp"].astype(np.float32) \
            + out_b[None, :]
    return out
